# revision 14
# baseline (speedup 1.0000x reference)
"""Trainium2 Bass kernel for nn_DecoderForLarge (sparse_attention).

Math (per batch b):
  probs = softmax(10*tanh(a*final_q @ M @ emb.T - dist/sqrt(2)) + mask)
where the multi-head mean collapses to a full H-dim inner product scaled by
1/NH, with M := Wq.T @ Wk folded into HxH matrices A,C on host; q_graph is a
pure function of the inputs and is computed on host too.

All layout work is host-side numpy: both emb orientations shipped fp16
(embT pre-scaled by ALPHA), the visited mask shipped transposed with 1/N
folded in, last-node embedding/coordinate gathers done on host. Distances
use a K=10 fp16 hi/lo-split matmul (exact to ~2^-22): d2 = c2 - 2*lc.c with
r2 folded into the Sqrt bias. On device only the O(G*N) compute remains:
  d2 matmul (K=10 fp16) -> Sqrt -> score matmul (K=128 fp16) -> Pool
  subtracts dist -> Tanh -> DVE *10+mask -> Exp(+accum) -> normalize
  -> fp16 out (host casts fp32).
The Act engine is the bottleneck (3 passes over G x N per batch); PSUM runs
two 2-buf rings of [128,1024] tiles (d2/pre and score) so Sqrt/Tanh always
have a ready tile, and Sqrts are batched so each act table loads once.

Sharding: data-parallel over batch B=32 -> 8 cores x 4 batches.
"""
import sys

sys.path.insert(0, "/opt/trn_rl_repo")

import numpy as np

import concourse.bass as bass
import concourse.tile as tile
from concourse import mybir


def _ensure_axon_hooks():
    """The image's antenv may lack axon_hooks, which bass_utils imports
    when trace=True under axon. Inject it and register the real NTFF
    profiling hook if the injected .so supports it."""
    try:
        import antenv.axon_hooks  # noqa: F401
        return
    except ImportError:
        pass
    import types
    import antenv

    mod = types.ModuleType("antenv.axon_hooks")
    mod._hook = None
    mod.set_axon_ntff_profile_hook = lambda h: setattr(mod, "_hook", h)
    mod.get_axon_ntff_profile_hook = lambda: mod._hook
    sys.modules["antenv.axon_hooks"] = mod
    antenv.axon_hooks = mod
    try:
        from trn_agent_boot.trn_boot import _ntff_profile_via_ctypes
        mod._hook = _ntff_profile_via_ctypes("/opt/axon/libaxon_pjrt.so")
    except Exception:
        mod._hook = None


_ensure_axon_hooks()

F32 = mybir.dt.float32
F16 = mybir.dt.float16

B, N, G, H, NH, D = 32, 2000, 200, 128, 8, 2
NCORES = 8
BPC = B // NCORES          # batches per core
NPAD = 2048                # N padded to 16*128
NCH = NPAD // 128          # column chunks
HC = 1024                  # PSUM tile width (2 banks)
GP = 256                   # G padded to 2*128
K10 = 10                   # hi/lo split distance-matmul contraction dim
ALPHA = 1.0 / (NH * np.sqrt(np.float64(H)))   # head-mean * 1/sqrt(H)
NEG_BIG = -60000.0         # fp16-safe; exp(10*tanh + NEG_BIG) == 0 exactly
D2_EPS = 3e-7              # covers fp32-accum noise so sqrt never sees <0
AF = mybir.ActivationFunctionType
OP = mybir.AluOpType


def build_nc() -> bass.Bass:
    nc = bass.Bass()

    negi_d = nc.dram_tensor("negi", [128, 128], F16, kind="ExternalInput")
    lhs_d = nc.dram_tensor("lhs10", [K10, BPC, GP], F16, kind="ExternalInput")
    coart_d = nc.dram_tensor("coart", [K10, BPC, NPAD], F16, kind="ExternalInput")
    bias_d = nc.dram_tensor("bias", [128, BPC, 2], F32, kind="ExternalInput")
    a_d = nc.dram_tensor("a16", [H, H], F16, kind="ExternalInput")
    c_d = nc.dram_tensor("c16", [H, H], F16, kind="ExternalInput")
    qg_d = nc.dram_tensor("qg", [128, BPC], F32, kind="ExternalInput")
    lnet_d = nc.dram_tensor("lnet", [128, BPC, GP], F16, kind="ExternalInput")
    embh_d = nc.dram_tensor("embh", [128, BPC, NCH, H], F16, kind="ExternalInput")
    mtc_d = nc.dram_tensor("masktc", [128, BPC, NCH, GP], F16, kind="ExternalInput")
    embt_d = nc.dram_tensor("embt", [128, BPC, NPAD], F16, kind="ExternalInput")
    gnm_d = nc.dram_tensor("gnm", [128, BPC, 2, NPAD], F16, kind="ExternalInput")
    out_d = nc.dram_tensor("probs", [128, BPC, 2, N], F16, kind="ExternalOutput")

    with tile.TileContext(nc) as tc:
        with (
            tc.tile_pool(name="consts", bufs=1) as consts,
            tc.tile_pool(name="dsp", bufs=2 * BPC) as dsp,
            tc.tile_pool(name="sm", bufs=2) as sm,
            tc.tile_pool(name="ew", bufs=2) as ew,
            tc.tile_pool(name="pp", bufs=2, space="PSUM") as pp,
        ):
            # ---------------- const loads (distance inputs first) --------
            lhs_s = consts.tile([K10, BPC, GP], F16)
            nc.sync.dma_start(out=lhs_s, in_=lhs_d[:, :, :])
            coart_s = consts.tile([K10, BPC, NPAD], F16)
            nc.sync.dma_start(out=coart_s, in_=coart_d[:, :, :])
            bias_s = consts.tile([128, BPC, 2], F32)
            nc.sync.dma_start(out=bias_s, in_=bias_d[:, :, :])
            negi_s = consts.tile([128, 128], F16)
            nc.sync.dma_start(out=negi_s, in_=negi_d[:, :])
            a_s = consts.tile([H, H], F16)
            nc.sync.dma_start(out=a_s, in_=a_d[:, :])
            c_s = consts.tile([H, H], F16)
            nc.sync.dma_start(out=c_s, in_=c_d[:, :])
            qg_s = consts.tile([128, BPC], F32)
            nc.sync.dma_start(out=qg_s, in_=qg_d[:, :])
            lnet_s = consts.tile([128, BPC, GP], F16)
            nc.sync.dma_start(out=lnet_s, in_=lnet_d[:, :, :])
            embh_s = consts.tile([128, BPC, NCH, H], F16)
            mtc_s = consts.tile([128, BPC, NCH, GP], F16)
            embt_s = consts.tile([128, BPC, NPAD], F16)
            gnm_s = consts.tile([128, BPC, 2, NPAD], F16)
            for ib in range(BPC):
                nc.sync.dma_start(out=embh_s[:, ib], in_=embh_d[:, ib])
                nc.sync.dma_start(out=mtc_s[:, ib], in_=mtc_d[:, ib])
                nc.sync.dma_start(out=embt_s[:, ib], in_=embt_d[:, ib])
                nc.sync.dma_start(out=gnm_s[:, ib], in_=gnm_d[:, ib])

            # half-tile column layout: [0:1024) and [1024:2000)
            HW2 = N - HC            # 976
            CSL = [(0, 512), (512, 512), (HC, 512), (HC + 512, HW2 - 512)]

            # ---------------- phase A: distances ----------------
            # d2 in half-tiles so Sqrt(k) overlaps the d2(k+1) matmuls.
            ds_all = {}
            for ib in range(BPC):
                for gt in range(2):
                    ds = dsp.tile([128, N], F16, tag="ds",
                                  name=f"ds_{ib}_{gt}")
                    for hf in range(2):
                        hw = HC if hf == 0 else HW2
                        t = pp.tile([128, hw], F32, tag="d2",
                                    padded_shape=[128, HC],
                                    name=f"d2_{ib}_{gt}_{hf}")
                        for o, w in CSL[hf * 2:hf * 2 + 2]:
                            nc.tensor.matmul(
                                t[:, o - hf * HC:o - hf * HC + w],
                                lhs_s[:, ib, gt * 128:(gt + 1) * 128],
                                coart_s[:, ib, o:o + w],
                                start=True, stop=True)
                        nc.scalar.activation(
                            out=ds[:, hf * HC:hf * HC + hw], in_=t,
                            func=AF.Sqrt, bias=bias_s[:, ib, gt:gt + 1],
                            scale=0.5)
                    ds_all[(ib, gt)] = ds

            # ---------------- phase B: score + softmax ----------------
            for ib in range(BPC):
                # pre-chain: vemb -> bank 0, qsum -> bank 1 (separate
                # accumulation groups). Batch 0 borrows the sc ring (idle
                # during phase A) so its chain runs under the Sqrts; later
                # batches use the then-idle d2 ring.
                pre = pp.tile([128, HC], F32, tag="sc" if ib == 0 else "d2",
                              name=f"pre_{ib}")
                for c in range(NCH):
                    nc.tensor.matmul(pre[:, 0:GP], embh_s[:, ib, c, :],
                                     mtc_s[:, ib, c, :],
                                     start=(c == 0), stop=(c == NCH - 1))
                vembt = sm.tile([H, GP], F16, tag="vembt", name=f"vembt_{ib}")
                nc.vector.tensor_copy(out=vembt, in_=pre[:, 0:GP])
                nc.tensor.matmul(pre[:, 512:768], a_s, lnet_s[:, ib, :],
                                 start=True, stop=False)
                nc.tensor.matmul(pre[:, 512:768], c_s, vembt,
                                 start=False, stop=True)
                qsumt = sm.tile([H, GP], F16, tag="qsumt", name=f"qsumt_{ib}")
                nc.vector.tensor_scalar(out=qsumt, in0=pre[:, 512:768],
                                        scalar1=qg_s[:, ib:ib + 1],
                                        scalar2=None, op0=OP.add)

                # all four Tanh halves first, then the two Exps, so the
                # Pool-side +mask passes hide under the next Tanh
                th_t = {}
                for gt in range(2):
                    ds = ds_all[(ib, gt)]
                    th = ew.tile([128, N], F16, tag="th",
                                 name=f"th_{ib}_{gt}")
                    for hf in range(2):
                        hw = HC if hf == 0 else HW2
                        sc = pp.tile([128, hw], F32, tag="sc",
                                     padded_shape=[128, HC],
                                     name=f"sc_{ib}_{gt}_{hf}")
                        for o, w in CSL[hf * 2:hf * 2 + 2]:
                            ol = o - hf * HC
                            nc.tensor.matmul(
                                sc[:, ol:ol + w],
                                qsumt[:, gt * 128:(gt + 1) * 128],
                                embt_s[:, ib, o:o + w],
                                start=True, stop=False)
                            nc.tensor.matmul(
                                sc[:, ol:ol + w], negi_s,
                                ds[:, o:o + w], start=False, stop=True)
                        hsl = slice(hf * HC, hf * HC + hw)
                        nc.scalar.activation(out=th[:, hsl], in_=sc,
                                             func=AF.Tanh)
                    # mask add on the idle Pool engine; the *10 rides on
                    # Exp's input scale (mask is pre-divided by 10 on host)
                    nc.gpsimd.tensor_tensor(out=th, in0=th,
                                            in1=gnm_s[:, ib, gt, 0:N],
                                            op=OP.add)
                    th_t[gt] = th
                for gt in range(2):
                    e = ew.tile([128, N], F16, tag="e", name=f"e_{ib}_{gt}")
                    esum = sm.tile([128, 1], F32, tag="esum",
                                   name=f"esum_{ib}_{gt}")
                    nc.scalar.activation(out=e, in_=th_t[gt], func=AF.Exp,
                                         scale=10.0, accum_out=esum[:, :])
                    nc.vector.reciprocal(out=esum, in_=esum)
                    nc.vector.tensor_scalar(out=e, in0=e,
                                            scalar1=esum[:, :], scalar2=None,
                                            op0=OP.mult)
                    nc.sync.dma_start(out=out_d[:, ib, gt, :], in_=e)
    return nc


def _split_multi_waits(bir: bytes, max_inline: int = 1) -> bytes:
    """This walrus build only accepts one inline sync-wait per instruction;
    Tile inlines many. Split extras into standalone EventSemaphore waits
    (same engine, immediately before), which is exactly the raw-bass form."""
    import orjson

    j = orjson.loads(bir)
    ctr = 0
    for fn in j["functions"]:
        for blk in fn["blocks"]:
            insts = blk.get("instructions")
            if not insts:
                continue
            out = []
            for inst in insts:
                si = inst.get("sync_info")
                waits = (si or {}).get("on_wait") or []
                if len(waits) > max_inline:
                    for w in waits[:-max_inline]:
                        ctr += 1
                        out.append({
                            "name": f"SW-{ctr}",
                            "opcode": "EventSemaphore",
                            "engine": inst["engine"],
                            "ins": [],
                            "outs": [],
                            "sync_info": {"on_wait": [w], "on_update": []},
                        })
                    si["on_wait"] = waits[-max_inline:]
                out.append(inst)
            blk["instructions"] = out
    return orjson.dumps(j)


_NC = None


def _get_nc():
    global _NC
    if _NC is None:
        _NC = build_nc()
        transformed = _split_multi_waits(_NC.to_json_bytes())
        _NC.to_json_bytes = lambda: transformed
    return _NC


def _split16(x32):
    """fp32 -> (hi, lo) fp16 pair with hi + lo ~= x to ~2^-22."""
    hi = x32.astype(np.float16)
    lo = (x32 - hi.astype(np.float32)).astype(np.float16)
    return hi, lo


def make_in_maps(embeddings, coordinates, last_node, group_ninf_mask,
                 Wq_graph, Wq_first, Wq_last, Wq, W_visited, Wk):
    """All layout/gather prep on host; returns 8 per-core input maps."""
    emb = np.asarray(embeddings, np.float32)
    coord = np.asarray(coordinates, np.float32)
    lastn = np.asarray(last_node).astype(np.int64)
    visited = np.isneginf(np.asarray(group_ninf_mask))      # (B, G, N) bool

    # --- weight products (fp64); q_graph fully host-side ---
    M = np.asarray(Wq, np.float64).T @ np.asarray(Wk, np.float64)
    wlf = (np.asarray(Wq_last, np.float64) + np.asarray(Wq_first, np.float64))
    a16 = np.ascontiguousarray((wlf.T @ M), np.float16)
    c16 = np.ascontiguousarray(np.asarray(W_visited, np.float64).T @ M,
                               np.float16)
    mean_emb = emb.astype(np.float64).mean(axis=1)          # (B, H)
    qg = np.ascontiguousarray((M.T @ np.asarray(Wq_graph, np.float64)
                               @ mean_emb.T), np.float32)   # (128, B)

    # --- emb, both orientations, fp16 ---
    emb_p = np.zeros((B, NPAD, H), np.float16)
    emb_p[:, :N] = emb
    embh = np.ascontiguousarray(
        emb_p.reshape(B, NCH, 128, H).transpose(2, 0, 1, 3))  # (128,B,NCH,H)
    embt = np.zeros((B, H, NPAD), np.float16)
    embt[:, :, :N] = (emb.transpose(0, 2, 1) * np.float32(ALPHA))
    embt = np.ascontiguousarray(embt.transpose(1, 0, 2))      # (128,B,NPAD)

    # --- transposed visited mask with 1/N folded in ---
    mtc = np.zeros((B, NPAD, GP), np.float16)
    mtc[:, :N, :G] = visited.transpose(0, 2, 1) * np.float32(1.0 / N)
    mtc = np.ascontiguousarray(
        mtc.reshape(B, NCH, 128, GP).transpose(2, 0, 1, 3))   # (128,B,NCH,GP)

    # --- additive mask, g-partition orientation, pre-divided by 10
    # (the *10 is applied via the Exp activation's input scale) ---
    mval = np.float16(NEG_BIG / 10.0)
    gnm = np.full((B, GP, NPAD), mval, np.float16)
    gnm[:, :G, :N] = np.where(visited, mval, np.float16(0.0))
    gnm = np.ascontiguousarray(
        gnm.reshape(B, 2, 128, NPAD).transpose(2, 0, 1, 3))   # (128,B,2,NPAD)

    # --- distance operands: hi/lo split coords (K=10 exact expansion) ---
    xh, xl = _split16(coord[:, :, 0])
    yh, yl = _split16(coord[:, :, 1])
    x64 = xh.astype(np.float64) + xl.astype(np.float64)
    y64 = yh.astype(np.float64) + yl.astype(np.float64)
    c2 = x64 * x64 + y64 * y64
    c2h = c2.astype(np.float16)
    c2l = (c2 - c2h.astype(np.float64)).astype(np.float16)
    coart = np.zeros((K10, B, NPAD), np.float16)
    for k, row in enumerate((c2h, c2l, xh, xh, xl, xl, yh, yh, yl, yl)):
        coart[k, :, :N] = row

    lastn_p = np.zeros((B, GP), np.int64)
    lastn_p[:, :G] = lastn
    bidx = np.arange(B)[:, None]
    lc = coord[bidx, lastn_p]                                 # (B, GP, 2)
    lxh, lxl = _split16(lc[:, :, 0])
    lyh, lyl = _split16(lc[:, :, 1])
    lhs10 = np.stack([
        np.ones((B, GP), np.float16), np.ones((B, GP), np.float16),
        -2.0 * lxh, -2.0 * lxl, -2.0 * lxh, -2.0 * lxl,
        -2.0 * lyh, -2.0 * lyl, -2.0 * lyh, -2.0 * lyl,
    ]).astype(np.float16)                                     # (K10, B, GP)
    lx64 = lxh.astype(np.float64) + lxl.astype(np.float64)
    ly64 = lyh.astype(np.float64) + lyl.astype(np.float64)
    r2 = lx64 * lx64 + ly64 * ly64
    bias = (0.5 * r2 + D2_EPS).astype(np.float32)             # (B, GP)
    bias = np.ascontiguousarray(
        bias.reshape(B, 2, 128).transpose(2, 0, 1), np.float32)  # (128,B,2)

    # --- host-gathered last-node embeddings, transposed ---
    lnet = np.ascontiguousarray(
        emb[bidx, lastn_p].astype(np.float16).transpose(2, 0, 1))  # (128,B,GP)

    negi = np.ascontiguousarray(-np.eye(128, dtype=np.float16))
    shared = {"a16": a16, "c16": c16, "negi": negi}
    in_maps = []
    for i in range(NCORES):
        sl = slice(i * BPC, (i + 1) * BPC)
        m = {
            "lhs10": np.ascontiguousarray(lhs10[:, sl]),
            "coart": np.ascontiguousarray(coart[:, sl]),
            "bias": np.ascontiguousarray(bias[:, sl]),
            "qg": np.ascontiguousarray(qg[:, sl]),
            "lnet": np.ascontiguousarray(lnet[:, sl]),
            "embh": np.ascontiguousarray(embh[:, sl]),
            "masktc": np.ascontiguousarray(mtc[:, sl]),
            "embt": np.ascontiguousarray(embt[:, sl]),
            "gnm": np.ascontiguousarray(gnm[:, sl]),
        }
        m.update(shared)
        in_maps.append(m)
    return in_maps


def kernel(embeddings, coordinates, last_node, group_ninf_mask, S,
           Wq_graph, Wq_first, Wq_last, Wq, W_visited, Wk, **run_kwargs):
    from concourse.bass_utils import run_bass_kernel_spmd

    nc = _get_nc()
    in_maps = make_in_maps(
        embeddings, coordinates, last_node, group_ninf_mask,
        Wq_graph, Wq_first, Wq_last, Wq, W_visited, Wk)
    res = run_bass_kernel_spmd(nc, in_maps, core_ids=list(range(NCORES)),
                               **run_kwargs)
    # (128, BPC, 2, N) fp16 per core -> (B, G, N) fp32
    parts = []
    for r in res.results:
        o = r["probs"].transpose(1, 2, 0, 3).reshape(BPC, GP, N)
        parts.append(o[:, :G].astype(np.float32))
    out = np.concatenate(parts, axis=0)
    kernel.last_results = res
    return out


# revision 16
# speedup vs baseline: 1.2034x; 1.2034x over previous
"""Trainium2 Bass kernel for nn_DecoderForLarge (sparse_attention).

Math (per batch b):
  probs = softmax(10*tanh(a*final_q @ M @ emb.T - dist/sqrt(2)) + mask)
where the multi-head mean collapses to a full H-dim inner product scaled by
1/NH, with M := Wq.T @ Wk folded into HxH matrices A,C on host; q_graph is a
pure function of the inputs and is computed on host too.

All layout work is host-side numpy: both emb orientations shipped fp16
(embT pre-scaled by ALPHA), the visited mask shipped transposed with 1/N
folded in, last-node embedding/coordinate gathers done on host. Distances
use a K=10 fp16 hi/lo-split matmul (exact to ~2^-22): d2 = c2 - 2*lc.c with
r2 folded into the Sqrt bias. On device only the O(G*N) compute remains:
  d2 matmul (K=10 fp16) -> Sqrt -> score matmul (K=128 fp16) -> Pool
  subtracts dist -> Tanh -> DVE *10+mask -> Exp(+accum) -> normalize
  -> fp16 out (host casts fp32).
The Act engine is the bottleneck (3 passes over G x N per batch); PSUM runs
two 2-buf rings of [128,1024] tiles (d2/pre and score) so Sqrt/Tanh always
have a ready tile, and Sqrts are batched so each act table loads once.

Sharding: data-parallel over batch B=32 -> 8 cores x 4 batches.
"""
import sys

sys.path.insert(0, "/opt/trn_rl_repo")

import numpy as np

import concourse.bass as bass
import concourse.tile as tile
from concourse import mybir


def _ensure_axon_hooks():
    """The image's antenv may lack axon_hooks, which bass_utils imports
    when trace=True under axon. Inject it and register the real NTFF
    profiling hook if the injected .so supports it."""
    try:
        import antenv.axon_hooks  # noqa: F401
        return
    except ImportError:
        pass
    import types
    import antenv

    mod = types.ModuleType("antenv.axon_hooks")
    mod._hook = None
    mod.set_axon_ntff_profile_hook = lambda h: setattr(mod, "_hook", h)
    mod.get_axon_ntff_profile_hook = lambda: mod._hook
    sys.modules["antenv.axon_hooks"] = mod
    antenv.axon_hooks = mod
    try:
        from trn_agent_boot.trn_boot import _ntff_profile_via_ctypes
        mod._hook = _ntff_profile_via_ctypes("/opt/axon/libaxon_pjrt.so")
    except Exception:
        mod._hook = None


_ensure_axon_hooks()

F32 = mybir.dt.float32
F16 = mybir.dt.float16

B, N, G, H, NH, D = 32, 2000, 200, 128, 8, 2
NCORES = 8
BPC = B // NCORES          # batches per core
NPAD = 2048                # N padded to 16*128
NCH = NPAD // 128          # column chunks
HC = 1024                  # PSUM tile width (2 banks)
GP = 256                   # G padded to 2*128
K10 = 10                   # hi/lo split distance-matmul contraction dim
ALPHA = 1.0 / (NH * np.sqrt(np.float64(H)))   # head-mean * 1/sqrt(H)
NEG_BIG = -60000.0         # fp16-safe; exp(10*tanh + NEG_BIG) == 0 exactly
D2_EPS = 3e-7              # covers fp32-accum noise so sqrt never sees <0
AF = mybir.ActivationFunctionType
OP = mybir.AluOpType


def build_nc() -> bass.Bass:
    nc = bass.Bass()

    negi_d = nc.dram_tensor("negi", [128, 128], F16, kind="ExternalInput")
    lhs_d = nc.dram_tensor("lhs10", [K10, BPC, GP], F16, kind="ExternalInput")
    coart_d = nc.dram_tensor("coart", [K10, BPC, NPAD], F16, kind="ExternalInput")
    bias_d = nc.dram_tensor("bias", [128, BPC, 2], F32, kind="ExternalInput")
    a_d = nc.dram_tensor("a16", [H, H], F16, kind="ExternalInput")
    c_d = nc.dram_tensor("c16", [H, H], F16, kind="ExternalInput")
    qg_d = nc.dram_tensor("qg", [128, BPC], F32, kind="ExternalInput")
    lnet_d = nc.dram_tensor("lnet", [128, BPC, GP], F16, kind="ExternalInput")
    embh_d = nc.dram_tensor("embh", [128, BPC, NCH, H], F16, kind="ExternalInput")
    mtc_d = nc.dram_tensor("masktc", [128, BPC, NCH, GP], F16, kind="ExternalInput")
    embt_d = nc.dram_tensor("embt", [128, BPC, NPAD], F16, kind="ExternalInput")
    gnm_d = nc.dram_tensor("gnm", [128, BPC, 2, NPAD], F16, kind="ExternalInput")
    out_d = nc.dram_tensor("probs", [128, BPC, 2, N], F16, kind="ExternalOutput")

    with tile.TileContext(nc) as tc:
        with (
            tc.tile_pool(name="consts", bufs=1) as consts,
            tc.tile_pool(name="dsp", bufs=2 * BPC) as dsp,
            tc.tile_pool(name="sm", bufs=2) as sm,
            tc.tile_pool(name="ew", bufs=2) as ew,
            tc.tile_pool(name="pp", bufs=2, space="PSUM") as pp,
        ):
            # ---------------- const loads (distance inputs first) --------
            lhs_s = consts.tile([K10, BPC, GP], F16)
            nc.sync.dma_start(out=lhs_s, in_=lhs_d[:, :, :])
            coart_s = consts.tile([K10, BPC, NPAD], F16)
            nc.sync.dma_start(out=coart_s, in_=coart_d[:, :, :])
            bias_s = consts.tile([128, BPC, 2], F32)
            nc.sync.dma_start(out=bias_s, in_=bias_d[:, :, :])
            negi_s = consts.tile([128, 128], F16)
            nc.sync.dma_start(out=negi_s, in_=negi_d[:, :])
            a_s = consts.tile([H, H], F16)
            nc.sync.dma_start(out=a_s, in_=a_d[:, :])
            c_s = consts.tile([H, H], F16)
            nc.sync.dma_start(out=c_s, in_=c_d[:, :])
            qg_s = consts.tile([128, BPC], F32)
            nc.sync.dma_start(out=qg_s, in_=qg_d[:, :])
            lnet_s = consts.tile([128, BPC, GP], F16)
            nc.sync.dma_start(out=lnet_s, in_=lnet_d[:, :, :])
            embh_s = consts.tile([128, BPC, NCH, H], F16)
            mtc_s = consts.tile([128, BPC, NCH, GP], F16)
            embt_s = consts.tile([128, BPC, NPAD], F16)
            gnm_s = consts.tile([128, BPC, 2, NPAD], F16)
            for ib in range(BPC):
                nc.sync.dma_start(out=embh_s[:, ib], in_=embh_d[:, ib])
                nc.sync.dma_start(out=mtc_s[:, ib], in_=mtc_d[:, ib])
                nc.sync.dma_start(out=embt_s[:, ib], in_=embt_d[:, ib])
                nc.sync.dma_start(out=gnm_s[:, ib], in_=gnm_d[:, ib])

            # half-tile column layout: [0:1024) and [1024:2000)
            HW2 = N - HC            # 976
            CSL = [(0, 512), (512, 512), (HC, 512), (HC + 512, HW2 - 512)]

            # ---------------- phase A: distances ----------------
            # d2 in half-tiles so Sqrt(k) overlaps the d2(k+1) matmuls.
            ds_all = {}
            for ib in range(BPC):
                for gt in range(2):
                    ds = dsp.tile([128, N], F16, tag="ds",
                                  name=f"ds_{ib}_{gt}")
                    for hf in range(2):
                        hw = HC if hf == 0 else HW2
                        t = pp.tile([128, hw], F32, tag="d2",
                                    padded_shape=[128, HC],
                                    name=f"d2_{ib}_{gt}_{hf}")
                        for o, w in CSL[hf * 2:hf * 2 + 2]:
                            nc.tensor.matmul(
                                t[:, o - hf * HC:o - hf * HC + w],
                                lhs_s[:, ib, gt * 128:(gt + 1) * 128],
                                coart_s[:, ib, o:o + w],
                                start=True, stop=True)
                        nc.scalar.activation(
                            out=ds[:, hf * HC:hf * HC + hw], in_=t,
                            func=AF.Sqrt, bias=bias_s[:, ib, gt:gt + 1],
                            scale=0.5)
                    ds_all[(ib, gt)] = ds

            # ---------------- phase B: score + softmax ----------------
            for ib in range(BPC):
                # pre-chain: vemb -> bank 0, qsum -> bank 1 (separate
                # accumulation groups). Batch 0 borrows the sc ring (idle
                # during phase A) so its chain runs under the Sqrts; later
                # batches use the then-idle d2 ring.
                pre = pp.tile([128, HC], F32, tag="d2", name=f"pre_{ib}")
                for c in range(NCH):
                    nc.tensor.matmul(pre[:, 0:GP], embh_s[:, ib, c, :],
                                     mtc_s[:, ib, c, :],
                                     start=(c == 0), stop=(c == NCH - 1))
                vembt = sm.tile([H, GP], F16, tag="vembt", name=f"vembt_{ib}")
                nc.vector.tensor_copy(out=vembt, in_=pre[:, 0:GP])
                nc.tensor.matmul(pre[:, 512:768], a_s, lnet_s[:, ib, :],
                                 start=True, stop=False)
                nc.tensor.matmul(pre[:, 512:768], c_s, vembt,
                                 start=False, stop=True)
                qsumt = sm.tile([H, GP], F16, tag="qsumt", name=f"qsumt_{ib}")
                nc.vector.tensor_scalar(out=qsumt, in0=pre[:, 512:768],
                                        scalar1=qg_s[:, ib:ib + 1],
                                        scalar2=None, op0=OP.add)

                # all four Tanh halves first, then the two Exps, so the
                # Pool-side +mask passes hide under the next Tanh
                th_t = {}
                for gt in range(2):
                    ds = ds_all[(ib, gt)]
                    th = ew.tile([128, N], F16, tag="th",
                                 name=f"th_{ib}_{gt}")
                    for hf in range(2):
                        hw = HC if hf == 0 else HW2
                        sc = pp.tile([128, hw], F32, tag="sc",
                                     padded_shape=[128, HC],
                                     name=f"sc_{ib}_{gt}_{hf}")
                        for o, w in CSL[hf * 2:hf * 2 + 2]:
                            ol = o - hf * HC
                            nc.tensor.matmul(
                                sc[:, ol:ol + w],
                                qsumt[:, gt * 128:(gt + 1) * 128],
                                embt_s[:, ib, o:o + w],
                                start=True, stop=False)
                            nc.tensor.matmul(
                                sc[:, ol:ol + w], negi_s,
                                ds[:, o:o + w], start=False, stop=True)
                        hsl = slice(hf * HC, hf * HC + hw)
                        nc.scalar.activation(out=th[:, hsl], in_=sc,
                                             func=AF.Tanh)
                        # mask add per half, split across the idle Pool
                        # (h0, hides under Tanh h1) and DVE (h1); the *10
                        # rides on Exp's input scale (mask pre-divided
                        # by 10 on host)
                        eng = nc.gpsimd if hf == 0 else nc.vector
                        eng.tensor_tensor(out=th[:, hsl], in0=th[:, hsl],
                                          in1=gnm_s[:, ib, gt, hsl],
                                          op=OP.add)
                    th_t[gt] = th
                for gt in range(2):
                    e = ew.tile([128, N], F16, tag="e", name=f"e_{ib}_{gt}")
                    esum = sm.tile([128, 1], F32, tag="esum",
                                   name=f"esum_{ib}_{gt}")
                    nc.scalar.activation(out=e, in_=th_t[gt], func=AF.Exp,
                                         scale=10.0, accum_out=esum[:, :])
                    nc.vector.reciprocal(out=esum, in_=esum)
                    nc.vector.tensor_scalar(out=e, in0=e,
                                            scalar1=esum[:, :], scalar2=None,
                                            op0=OP.mult)
                    nc.sync.dma_start(out=out_d[:, ib, gt, :], in_=e)
    return nc


def _split_multi_waits(bir: bytes, max_inline: int = 1) -> bytes:
    """This walrus build only accepts one inline sync-wait per instruction;
    Tile inlines many. Split extras into standalone EventSemaphore waits
    (same engine, immediately before), which is exactly the raw-bass form."""
    import orjson

    j = orjson.loads(bir)
    ctr = 0
    for fn in j["functions"]:
        for blk in fn["blocks"]:
            insts = blk.get("instructions")
            if not insts:
                continue
            out = []
            for inst in insts:
                si = inst.get("sync_info")
                waits = (si or {}).get("on_wait") or []
                if len(waits) > max_inline:
                    for w in waits[:-max_inline]:
                        ctr += 1
                        out.append({
                            "name": f"SW-{ctr}",
                            "opcode": "EventSemaphore",
                            "engine": inst["engine"],
                            "ins": [],
                            "outs": [],
                            "sync_info": {"on_wait": [w], "on_update": []},
                        })
                    si["on_wait"] = waits[-max_inline:]
                out.append(inst)
            blk["instructions"] = out
    return orjson.dumps(j)


_NC = None


def _get_nc():
    global _NC
    if _NC is None:
        _NC = build_nc()
        transformed = _split_multi_waits(_NC.to_json_bytes())
        _NC.to_json_bytes = lambda: transformed
    return _NC


def _split16(x32):
    """fp32 -> (hi, lo) fp16 pair with hi + lo ~= x to ~2^-22."""
    hi = x32.astype(np.float16)
    lo = (x32 - hi.astype(np.float32)).astype(np.float16)
    return hi, lo


def make_in_maps(embeddings, coordinates, last_node, group_ninf_mask,
                 Wq_graph, Wq_first, Wq_last, Wq, W_visited, Wk):
    """All layout/gather prep on host; returns 8 per-core input maps."""
    emb = np.asarray(embeddings, np.float32)
    coord = np.asarray(coordinates, np.float32)
    lastn = np.asarray(last_node).astype(np.int64)
    visited = np.isneginf(np.asarray(group_ninf_mask))      # (B, G, N) bool

    # --- weight products (fp64); q_graph fully host-side ---
    M = np.asarray(Wq, np.float64).T @ np.asarray(Wk, np.float64)
    wlf = (np.asarray(Wq_last, np.float64) + np.asarray(Wq_first, np.float64))
    a16 = np.ascontiguousarray((wlf.T @ M), np.float16)
    c16 = np.ascontiguousarray(np.asarray(W_visited, np.float64).T @ M,
                               np.float16)
    mean_emb = emb.astype(np.float64).mean(axis=1)          # (B, H)
    qg = np.ascontiguousarray((M.T @ np.asarray(Wq_graph, np.float64)
                               @ mean_emb.T), np.float32)   # (128, B)

    # --- emb, both orientations, fp16 ---
    emb_p = np.zeros((B, NPAD, H), np.float16)
    emb_p[:, :N] = emb
    embh = np.ascontiguousarray(
        emb_p.reshape(B, NCH, 128, H).transpose(2, 0, 1, 3))  # (128,B,NCH,H)
    embt = np.zeros((B, H, NPAD), np.float16)
    embt[:, :, :N] = (emb.transpose(0, 2, 1) * np.float32(ALPHA))
    embt = np.ascontiguousarray(embt.transpose(1, 0, 2))      # (128,B,NPAD)

    # --- transposed visited mask with 1/N folded in ---
    mtc = np.zeros((B, NPAD, GP), np.float16)
    mtc[:, :N, :G] = visited.transpose(0, 2, 1) * np.float32(1.0 / N)
    mtc = np.ascontiguousarray(
        mtc.reshape(B, NCH, 128, GP).transpose(2, 0, 1, 3))   # (128,B,NCH,GP)

    # --- additive mask, g-partition orientation, pre-divided by 10
    # (the *10 is applied via the Exp activation's input scale) ---
    mval = np.float16(NEG_BIG / 10.0)
    gnm = np.full((B, GP, NPAD), mval, np.float16)
    gnm[:, :G, :N] = np.where(visited, mval, np.float16(0.0))
    gnm = np.ascontiguousarray(
        gnm.reshape(B, 2, 128, NPAD).transpose(2, 0, 1, 3))   # (128,B,2,NPAD)

    # --- distance operands: hi/lo split coords (K=10 exact expansion) ---
    xh, xl = _split16(coord[:, :, 0])
    yh, yl = _split16(coord[:, :, 1])
    x64 = xh.astype(np.float64) + xl.astype(np.float64)
    y64 = yh.astype(np.float64) + yl.astype(np.float64)
    c2 = x64 * x64 + y64 * y64
    c2h = c2.astype(np.float16)
    c2l = (c2 - c2h.astype(np.float64)).astype(np.float16)
    coart = np.zeros((K10, B, NPAD), np.float16)
    for k, row in enumerate((c2h, c2l, xh, xh, xl, xl, yh, yh, yl, yl)):
        coart[k, :, :N] = row

    lastn_p = np.zeros((B, GP), np.int64)
    lastn_p[:, :G] = lastn
    bidx = np.arange(B)[:, None]
    lc = coord[bidx, lastn_p]                                 # (B, GP, 2)
    lxh, lxl = _split16(lc[:, :, 0])
    lyh, lyl = _split16(lc[:, :, 1])
    lhs10 = np.stack([
        np.ones((B, GP), np.float16), np.ones((B, GP), np.float16),
        -2.0 * lxh, -2.0 * lxl, -2.0 * lxh, -2.0 * lxl,
        -2.0 * lyh, -2.0 * lyl, -2.0 * lyh, -2.0 * lyl,
    ]).astype(np.float16)                                     # (K10, B, GP)
    lx64 = lxh.astype(np.float64) + lxl.astype(np.float64)
    ly64 = lyh.astype(np.float64) + lyl.astype(np.float64)
    r2 = lx64 * lx64 + ly64 * ly64
    bias = (0.5 * r2 + D2_EPS).astype(np.float32)             # (B, GP)
    bias = np.ascontiguousarray(
        bias.reshape(B, 2, 128).transpose(2, 0, 1), np.float32)  # (128,B,2)

    # --- host-gathered last-node embeddings, transposed ---
    lnet = np.ascontiguousarray(
        emb[bidx, lastn_p].astype(np.float16).transpose(2, 0, 1))  # (128,B,GP)

    negi = np.ascontiguousarray(-np.eye(128, dtype=np.float16))
    shared = {"a16": a16, "c16": c16, "negi": negi}
    in_maps = []
    for i in range(NCORES):
        sl = slice(i * BPC, (i + 1) * BPC)
        m = {
            "lhs10": np.ascontiguousarray(lhs10[:, sl]),
            "coart": np.ascontiguousarray(coart[:, sl]),
            "bias": np.ascontiguousarray(bias[:, sl]),
            "qg": np.ascontiguousarray(qg[:, sl]),
            "lnet": np.ascontiguousarray(lnet[:, sl]),
            "embh": np.ascontiguousarray(embh[:, sl]),
            "masktc": np.ascontiguousarray(mtc[:, sl]),
            "embt": np.ascontiguousarray(embt[:, sl]),
            "gnm": np.ascontiguousarray(gnm[:, sl]),
        }
        m.update(shared)
        in_maps.append(m)
    return in_maps


def kernel(embeddings, coordinates, last_node, group_ninf_mask, S,
           Wq_graph, Wq_first, Wq_last, Wq, W_visited, Wk, **run_kwargs):
    from concourse.bass_utils import run_bass_kernel_spmd

    nc = _get_nc()
    in_maps = make_in_maps(
        embeddings, coordinates, last_node, group_ninf_mask,
        Wq_graph, Wq_first, Wq_last, Wq, W_visited, Wk)
    res = run_bass_kernel_spmd(nc, in_maps, core_ids=list(range(NCORES)),
                               **run_kwargs)
    # (128, BPC, 2, N) fp16 per core -> (B, G, N) fp32
    parts = []
    for r in res.results:
        o = r["probs"].transpose(1, 2, 0, 3).reshape(BPC, GP, N)
        parts.append(o[:, :G].astype(np.float32))
    out = np.concatenate(parts, axis=0)
    kernel.last_results = res
    return out
